# revision 1
# baseline (speedup 1.0000x reference)
"""Trainium2 Bass kernel for nn_DebiasLoss: data-parallel mean cross-entropy
with class-prior margin and target-column dispersion margin.

Sharding: logits/targets split along batch across 8 NeuronCores; w_norm /
class_bias replicated; each core emits (sum of its row losses)/B and the host
adds the 8 partial scalars (the all-reduce of the hint).

Math per row r (t = target, BETA=0.5, LAMDA=1.0):
    mlf[c]   = log(class_bias[c] + 1e-12)
    rv[c]    = logits[r,c] + mlf[c]
    S0       = sum_c exp(rv[c])                     (ScalarE Exp + accumulator)
    keep     = any_c(logits[r,c] > logits[r,t])    (count via is_gt/relu accum)
    delta    = BETA * coef * keep * log1p((tgt/wn_t - wn_t)^2)
    S_adj    = S0 + exp(mlf[t] + tgt) * (exp(-delta) - 1)
    loss_r   = log(S_adj) - tgt - mlf[t] + delta
which equals logsumexp(adj) - adj[t] of the reference.

Layout trick: each core's 2048 rows are sorted by target on the host and
assigned to 16 row-tiles of 128 (row r = 128j + p), so tile j's targets all
fall inside a fixed 192-wide class window W_j.  The per-row gathers
(logits[r,t], w_norm[t], mlf[t]) then become cheap windowed iota-mask
scalar_tensor_tensor ops on VectorE instead of indirect DMAs.
"""

import os
from contextlib import ExitStack

import numpy as np

B, C = 16384, 1000
N_CORES = 8
R = B // N_CORES  # 2048 rows per core
P = 128           # SBUF partitions
T = R // P        # 16 row-tiles per core
W = 192           # class-window width per tile (margin ~6 sigma for uniform targets)
BETA = 0.5
LOG_EPS = 1e-12

# target-class window start for tile j (compile-time constants)
WIN = [max(0, min(C - W, round(62.5 * j - 64.75))) for j in range(T)]

# keep-count on VectorE for tiles [0, CNT_DVE), on ScalarE (relu) for the rest
CNT_DVE_TILES = int(os.environ.get("KRN_CNT_DVE", "7"))
# rv = logits + mlf on GpSimd (TensorTensor) for this many tiles, VectorE rest
RV_POOL_TILES = int(os.environ.get("KRN_RV_POOL", "2"))

_CACHE = {}


def _patch_act_tables():
    """Make every activation this kernel uses resolve to the single table set
    natural_log_exp_and_others (Exp, Ln, Relu, Identity, Copy, ...), so the
    compiler emits one ACT_TABLE_LOAD instead of thrashing between sets."""
    import concourse.hw_specs as hw_specs
    import concourse.bacc as bacc_mod

    if _CACHE.get("tables_patched"):
        return
    orig = hw_specs.get_activation_tables

    def filtered(module_arch):
        import concourse.mybir as mybir

        tabs = {k: set(v) for k, v in orig(module_arch).items()}
        keep_set = "natural_log_exp_and_others"
        ours = {
            mybir.ActivationFunctionType.Exp,
            mybir.ActivationFunctionType.Ln,
            mybir.ActivationFunctionType.Relu,
            mybir.ActivationFunctionType.Identity,
            mybir.ActivationFunctionType.Copy,
            mybir.ActivationFunctionType.Square,
        }
        assert ours <= tabs[keep_set]
        for name, fns in tabs.items():
            if name != keep_set:
                tabs[name] = fns - ours
        return tabs

    hw_specs.get_activation_tables = filtered
    bacc_mod.get_activation_tables = filtered
    _CACHE["tables_patched"] = True


def _build(wide=False, debug_taps=False):
    import concourse.bacc as bacc
    import concourse.bass as bass
    import concourse.tile as tile
    from concourse import mybir

    _patch_act_tables()

    f32 = mybir.dt.float32
    Alu = mybir.AluOpType
    Act = mybir.ActivationFunctionType
    X = mybir.AxisListType.X

    win = [0] * T if wide else WIN
    w_w = C if wide else W

    nc = bacc.Bacc(
        "TRN2",
        target_bir_lowering=False,
        debug=False,
        enable_asserts=False,
        num_devices=N_CORES,
    )

    d_logits = nc.dram_tensor("logits", [R, C], f32, kind="ExternalInput")
    d_trel = nc.dram_tensor("trel", [P, T], f32, kind="ExternalInput")
    d_iota = nc.dram_tensor("iota_w", [1, w_w], f32, kind="ExternalInput")
    # table-gather helpers: exact per-tile windows of width 128
    d_trel2 = nc.dram_tensor("trel2", [1, R], f32, kind="ExternalInput")
    d_wcw = nc.dram_tensor("wcw", [P, 2 * T], f32, kind="ExternalInput")
    d_pcol = nc.dram_tensor("p_col", [P, 1], f32, kind="ExternalInput")
    d_wrow = nc.dram_tensor("w_row", [1, C], f32, kind="ExternalInput")
    d_cb = nc.dram_tensor("cb_row", [1, C], f32, kind="ExternalInput")
    d_coef = nc.dram_tensor("coef", [1, 1], f32, kind="ExternalInput")
    d_out = nc.dram_tensor("out", [1, 1], f32, kind="ExternalOutput")
    d_dbg = {}
    if debug_taps:
        for nm, shp in [
            ("dbg_S0", [P, T]), ("dbg_cnt", [P, T]), ("dbg_tgt", [P, T]),
            ("dbg_wn", [P, T]), ("dbg_mt", [P, T]), ("dbg_delta", [P, T]),
            ("dbg_lossr", [P, T]),
        ]:
            d_dbg[nm] = nc.dram_tensor(nm, shp, f32, kind="ExternalOutput")

    with tile.TileContext(nc) as tc:
        with ExitStack() as ctx:
            big = ctx.enter_context(tc.tile_pool(name="big", bufs=6))
            rvp = ctx.enter_context(tc.tile_pool(name="rvp", bufs=4))
            one = ctx.enter_context(tc.tile_pool(name="one", bufs=1))
            sm = ctx.enter_context(tc.tile_pool(name="sm", bufs=1))
            psp = ctx.enter_context(tc.tile_pool(name="psp", bufs=1, space="PSUM"))

            # ---- one-time setup -------------------------------------------
            eps12 = sm.tile([P, 1], f32, tag="eps12")
            nc.vector.memset(eps12[:], LOG_EPS)

            # issue the first two logits-tile loads ahead of the prologue
            # broadcasts so compute ramps ~1.5us earlier
            lt_pre = {}
            for j0 in (0, 1):
                lt_t = big.tile([P, C], f32, tag="lt")
                nc.sync.dma_start(
                    out=lt_t[:], in_=d_logits.ap()[j0 * P : (j0 + 1) * P, :]
                )
                lt_pre[j0] = lt_t[:]

            cb_bc = one.tile([P, C], f32, tag="cb_bc")
            nc.sync.dma_start(out=cb_bc[:], in_=d_cb.ap().to_broadcast([P, C]))
            mlf = one.tile([P, C], f32, tag="mlf")
            nc.scalar.activation(out=mlf[:], in_=cb_bc[:], func=Act.Ln, bias=eps12[:])

            iota_w = one.tile([P, w_w], f32, tag="iota_w")
            nc.sync.dma_start(out=iota_w[:], in_=d_iota.ap().to_broadcast([P, w_w]))

            if wide:
                wn_bc = one.tile([P, C], f32, tag="wn_bc")
                nc.sync.dma_start(out=wn_bc[:], in_=d_wrow.ap().to_broadcast([P, C]))

            trel = sm.tile([P, T], f32, tag="trel")
            nc.sync.dma_start(out=trel[:], in_=d_trel.ap())

            # ---- main loop over 16 row-tiles ------------------------------
            S0 = sm.tile([P, T], f32, tag="S0")
            TGT = sm.tile([P, T], f32, tag="TGT")
            WN = sm.tile([P, T], f32, tag="WN")
            MT = sm.tile([P, T], f32, tag="MT")
            cntd = sm.tile([P, T], f32, tag="cntd")
            cnta = sm.tile([P, T], f32, tag="cnta")
            nc.vector.memset(cntd[:], 0.0)
            nc.scalar.memzero(cnta[:])
            garb_d = one.tile([P, C], f32, tag="garb_d")
            garb_a = one.tile([P, C], f32, tag="garb_a")
            garb_w = one.tile([P, w_w], f32, tag="garb_w")
            ep = psp.tile([P, C], f32, tag="ep")

            tg = {}

            def tg_dma():
                # maskT[c, r] = 1[c == t_r - c2_{j(r)}]
                trel2_bc = one.tile([P, R], f32, tag="trel2_bc")
                tg["trel2_bc"] = trel2_bc
                nc.sync.dma_start(
                    out=tg["trel2_bc"][:], in_=d_trel2.ap().to_broadcast([P, R])
                )
                p_col = sm.tile([P, 1], f32, tag="p_col")
                tg["p_col"] = p_col
                nc.sync.dma_start(out=tg["p_col"][:], in_=d_pcol.ap())
                wcw = one.tile([P, 2 * T], f32, tag="wcw")
                tg["wcw"] = wcw
                nc.sync.dma_start(out=tg["wcw"][:], in_=d_wcw.ap())
                nc.scalar.activation(
                    out=tg["wcw"][:].rearrange("p (t o) -> p t o", o=2)[:, :, 1],
                    in_=tg["wcw"][:].rearrange("p (t o) -> p t o", o=2)[:, :, 1],
                    func=Act.Ln, bias=eps12[:],
                )
                ps_g = psp.tile([P, 2 * T], f32, tag="ps_g")
                tg["ps_g"] = ps_g

            def tg_mask():
                maskT = one.tile([P, R], f32, tag="maskT")
                tg["maskT"] = maskT
                eng = nc.gpsimd if os.environ.get("KRN_MASK_POOL", "0") == "1" else nc.vector
                eng.tensor_tensor(
                    out=tg["maskT"][:], in0=tg["p_col"][:].to_broadcast([P, R]),
                    in1=tg["trel2_bc"][:], op=Alu.is_equal,
                )

            def tg_matmul(jj):
                # ps_g[r, 2j:2j+2] = sum_c maskT[c, 128j+r]*wcw[c, 2j:2j+2]
                nc.tensor.matmul(
                    out=tg["ps_g"][:, 2 * jj : 2 * jj + 2],
                    lhsT=tg["maskT"][:, jj * P : (jj + 1) * P],
                    rhs=tg["wcw"][:, 2 * jj : 2 * jj + 2],
                    start=True, stop=True,
                )

            for j in range(T):
                if not wide:
                    if j == 5:
                        tg_dma()
                    elif j == 8:
                        tg_mask()
                    if 8 <= j:
                        for jj in range(2 * (j - 8), 2 * (j - 7)):
                            tg_matmul(jj)
                if j in lt_pre:
                    lt = lt_pre[j]
                else:
                    lt_t = big.tile([P, C], f32, tag="lt")
                    nc.sync.dma_start(
                        out=lt_t[:], in_=d_logits.ap()[j * P : (j + 1) * P, :]
                    )
                    lt = lt_t[:]
                sl = slice(win[j], win[j] + w_w)
                tcol = trel[:, j : j + 1]

                # windowed gather of the target logit (VectorE iota-mask)
                nc.vector.scalar_tensor_tensor(
                    out=garb_w[:], in0=iota_w[:], scalar=tcol, in1=lt[:, sl],
                    op0=Alu.is_equal, op1=Alu.mult, accum_out=TGT[:, j : j + 1],
                )
                if wide:
                    nc.vector.scalar_tensor_tensor(
                        out=garb_w[:], in0=iota_w[:], scalar=tcol, in1=wn_bc[:, sl],
                        op0=Alu.is_equal, op1=Alu.mult, accum_out=WN[:, j : j + 1],
                    )
                    nc.vector.scalar_tensor_tensor(
                        out=garb_w[:], in0=iota_w[:], scalar=tcol, in1=mlf[:, sl],
                        op0=Alu.is_equal, op1=Alu.mult, accum_out=MT[:, j : j + 1],
                    )

                # rv = logits + mlf
                rv = rvp.tile([P, C], f32, tag="rv")
                if j >= T - RV_POOL_TILES:
                    nc.gpsimd.tensor_tensor(out=rv[:], in0=lt, in1=mlf[:], op=Alu.add)
                else:
                    nc.vector.scalar_tensor_tensor(
                        out=rv[:], in0=lt, scalar=0.0, in1=mlf[:],
                        op0=Alu.add, op1=Alu.add,
                    )
                nc.scalar.activation(
                    out=ep[:], in_=rv[:], func=Act.Exp, accum_out=S0[:, j : j + 1],
                )

                # keep-count
                if j < CNT_DVE_TILES:
                    nc.vector.tensor_scalar(
                        out=garb_d[:], in0=lt,
                        scalar1=TGT[:, j : j + 1], scalar2=None,
                        op0=Alu.is_gt, op1=Alu.add,
                        accum_out=cntd[:, j : j + 1],
                    )
                else:
                    negc = sm.tile([P, 1], f32, tag=f"negc{j}")
                    nc.vector.tensor_scalar_mul(negc[:], TGT[:, j : j + 1], -1.0)
                    nc.scalar.activation(
                        out=garb_a[:], in_=lt, func=Act.Relu,
                        bias=negc[:], accum_out=cnta[:, j : j + 1],
                    )

            if not wide:
                psv = tg["ps_g"][:].rearrange("p (t o) -> p t o", o=2)
                nc.vector.tensor_copy(WN[:], psv[:, :, 0])
                nc.vector.tensor_copy(MT[:], psv[:, :, 1])

            coefb = sm.tile([P, 1], f32, tag="coefb")
            nc.sync.dma_start(out=coefb[:], in_=d_coef.ap().to_broadcast([P, 1]))
            kbeta = sm.tile([P, 1], f32, tag="kbeta")
            nc.vector.tensor_scalar_mul(kbeta[:], coefb[:], BETA)

            # ---- per-row tail on [P, T] tiles -----------------------------
            cnt = sm.tile([P, T], f32, tag="cnt")
            nc.vector.tensor_tensor(out=cnt[:], in0=cntd[:], in1=cnta[:], op=Alu.add)

            rw = sm.tile([P, T], f32, tag="rw")
            nc.vector.reciprocal(rw[:], WN[:])
            t1 = sm.tile([P, T], f32, tag="t1")
            nc.vector.tensor_mul(t1[:], TGT[:], rw[:])
            q = sm.tile([P, T], f32, tag="q")
            nc.vector.tensor_tensor(out=q[:], in0=t1[:], in1=WN[:], op=Alu.subtract)
            qq = sm.tile([P, T], f32, tag="qq")
            nc.vector.tensor_mul(qq[:], q[:], q[:])
            d0 = sm.tile([P, T], f32, tag="d0")
            nc.scalar.activation(out=d0[:], in_=qq[:], func=Act.Ln, bias=1.0)

            kc = sm.tile([P, T], f32, tag="kc")
            nc.vector.tensor_scalar(
                out=kc[:], in0=cnt[:], scalar1=0.0, scalar2=kbeta[:, 0:1],
                op0=Alu.is_gt, op1=Alu.mult,
            )
            delta = sm.tile([P, T], f32, tag="delta")
            nc.vector.tensor_mul(delta[:], kc[:], d0[:])

            # u = exp(mlf[t] + tgt);  a2 = tgt + mlf[t]
            a2 = sm.tile([P, T], f32, tag="a2")
            nc.vector.tensor_tensor(out=a2[:], in0=TGT[:], in1=MT[:], op=Alu.add)
            u = sm.tile([P, T], f32, tag="u")
            nc.scalar.activation(out=u[:], in_=a2[:], func=Act.Exp)
            emd = sm.tile([P, T], f32, tag="emd")
            nc.scalar.activation(out=emd[:], in_=delta[:], func=Act.Exp, scale=-1.0)
            w_ = sm.tile([P, T], f32, tag="w_")
            nc.vector.scalar_tensor_tensor(
                out=w_[:], in0=emd[:], scalar=1.0, in1=u[:],
                op0=Alu.subtract, op1=Alu.mult,
            )
            sadj = sm.tile([P, T], f32, tag="sadj")
            nc.vector.tensor_tensor(out=sadj[:], in0=S0[:], in1=w_[:], op=Alu.add)
            lse = sm.tile([P, T], f32, tag="lse")
            nc.scalar.activation(out=lse[:], in_=sadj[:], func=Act.Ln)

            a1 = sm.tile([P, T], f32, tag="a1")
            nc.vector.tensor_tensor(out=a1[:], in0=lse[:], in1=delta[:], op=Alu.add)
            lossr = sm.tile([P, T], f32, tag="lossr")
            nc.vector.tensor_tensor(out=lossr[:], in0=a1[:], in1=a2[:], op=Alu.subtract)

            # ---- reduce 2048 row losses to one scalar ---------------------
            rowsum = sm.tile([P, 1], f32, tag="rowsum")
            nc.vector.reduce_sum(rowsum[:], lossr[:], axis=X)
            invb = sm.tile([P, 1], f32, tag="invb")
            nc.vector.memset(invb[:], 1.0 / B)
            ps = psp.tile([1, 1], f32, tag="ps")
            nc.tensor.matmul(out=ps[:], lhsT=rowsum[:], rhs=invb[:], start=True, stop=True)
            res = sm.tile([1, 1], f32, tag="res")
            nc.vector.tensor_copy(res[:], ps[:])
            nc.sync.dma_start(out=d_out.ap(), in_=res[:])

            if debug_taps:
                for nm, tl in [
                    ("dbg_S0", S0), ("dbg_cnt", cnt), ("dbg_tgt", TGT),
                    ("dbg_wn", WN), ("dbg_mt", MT), ("dbg_delta", delta),
                    ("dbg_lossr", lossr),
                ]:
                    nc.sync.dma_start(out=d_dbg[nm].ap(), in_=tl[:])

    nc.compile()
    return nc


def _get_nc(wide=False):
    key = "nc_wide" if wide else "nc"
    if key not in _CACHE:
        _CACHE[key] = _build(wide=wide)
    return _CACHE[key]


def _sort_core(ts):
    """Stable sort of a core's targets; returns (order, sorted, fits_windows)."""
    order = np.argsort(ts, kind="stable")
    ts_s = ts[order]
    tij = ts_s.reshape(T, P)
    lo, hi = tij.min(axis=1), tij.max(axis=1)
    fits = all(WIN[j] <= lo[j] and hi[j] < WIN[j] + W for j in range(T)) and bool(
        np.all(hi - lo < P)
    )
    return order, ts_s, fits


def _prep_in_maps(logits, targets, adaptive_marg_coef, w_norm, class_bias):
    logits = np.asarray(logits, dtype=np.float32)
    assert logits.shape == (B, C), logits.shape
    t = np.asarray(targets).astype(np.int64).ravel()
    w = np.asarray(w_norm, dtype=np.float32).ravel()
    cb = np.asarray(class_bias, dtype=np.float32).ravel()
    coef = np.asarray(adaptive_marg_coef, dtype=np.float32).reshape(())

    cb_row = np.ascontiguousarray(cb.reshape(1, C))
    coef_arr = np.full((1, 1), coef, dtype=np.float32)
    p_col = np.arange(P, dtype=np.float32).reshape(P, 1)

    per_core = []
    all_fit = True
    for k in range(N_CORES):
        sl = slice(k * R, (k + 1) * R)
        order, ts_s, fits = _sort_core(t[sl])
        all_fit = all_fit and fits
        per_core.append((np.ascontiguousarray(logits[sl][order]), ts_s))

    wide = not all_fit
    w_w = C if wide else W
    win = np.asarray([0] * T if wide else WIN, dtype=np.int64)
    iota = np.arange(w_w, dtype=np.float32).reshape(1, w_w)

    in_maps = []
    for logits_s, ts_s in per_core:
        # row r = 128j + p  ->  [P, T] with column j = tile j
        tpt = ts_s.reshape(T, P).T
        # exact 128-wide windows for the table gathers
        c2 = np.minimum(ts_s.reshape(T, P).min(axis=1), C - P)  # [T]
        trel2 = (ts_s - np.repeat(c2, P)).astype(np.float32).reshape(1, R)
        idx = (c2[None, :] + np.arange(P)[:, None]).astype(np.int64)  # [P, T]
        wcw = np.empty((P, 2 * T), dtype=np.float32)
        wcw[:, 0::2] = w[idx]
        wcw[:, 1::2] = cb[idx]
        in_maps.append(
            {
                "logits": logits_s,
                "trel": np.ascontiguousarray(
                    (tpt - win[None, :]).astype(np.float32)
                ),
                "trel2": trel2,
                "wcw": wcw,
                "p_col": p_col,
                "iota_w": iota,
                "w_row": np.ascontiguousarray(w.reshape(1, C)),
                "cb_row": cb_row,
                "coef": coef_arr,
            }
        )
    return in_maps, wide


def _run(inputs, trace=False):
    from concourse import bass_utils

    in_maps, wide = _prep_in_maps(**inputs)
    nc = _get_nc(wide=wide)
    res = bass_utils.run_bass_kernel_spmd(
        nc, in_maps, core_ids=list(range(N_CORES)), trace=trace
    )
    total = sum(float(r["out"][0, 0]) for r in res.results)
    return np.float32(total), res


def kernel(**inputs) -> np.ndarray:
    loss, _ = _run(inputs, trace=False)
    return loss

